# revision 1
# baseline (speedup 1.0000x reference)
# GQA attention layer (B=1, S=2048, HID=2560, H=32, HKV=8, D=128) on 8 TRN2
# NeuronCores. Tensor-parallel over kv-head groups: core c owns kv head c and
# its 4 query heads (Wq/Wk/Wv row shards, Wo column shard). The o_proj
# partials are combined with an on-device ReduceScatter over the sequence
# axis (4 chunks, overlapped with compute); the host reassembles the
# sequence-sharded outputs.
#
# Per-core dataflow (all matmuls bf16 -> fp32 PSUM):
#   1. QKV projection from X^T tiles (s-major output layout), per-head
#      RMSNorm + RoPE on DVE, PE-transpose of Q/K into [d, s] layout.
#   2. Scores are computed transposed (S^T[k, q] = K Q^T) so that the
#      P^T @ V matmul needs no transpose of the 16.8M-element prob matrix.
#      exp() on the scalar engine (no max subtraction: |scores| is bounded).
#      Softmax denominators via an ALL-ONES [128,128] stationary matmul on
#      the PE: every output partition receives the same column sums, so the
#      denominator arrives pre-broadcast and one DVE reciprocal + multiply
#      normalizes the PV output.
#   3. o_proj per 512-row chunk, ReduceScatter per 256-row half in bf16
#      (overlapped with the next chunk's compute), host upcasts/reorders.
import sys

if "/opt/trn_rl_repo" not in sys.path:
    sys.path.insert(0, "/opt/trn_rl_repo")

import numpy as np
import ml_dtypes

import concourse.bacc as bacc
import concourse.mybir as mybir
import concourse.tile as tile
from concourse import bass_utils, masks

BF16 = mybir.dt.bfloat16
F32 = mybir.dt.float32

B, S, HID = 1, 2048, 2560
H, HKV, D = 32, 8, 128
G = H // HKV  # q heads per kv head (= per core)
NC = 8  # cores
DQ = G * D  # per-core q width (512)
EPS = 1e-6
SCALE = 1.0 / float(np.sqrt(D))

ST = 128          # s positions per compute tile
N_ST = S // ST    # 16
HC = HID // 128   # 20 contraction chunks
XL = 256          # s positions per X^T DMA load tile
N_XL = S // XL    # 8
QC = 512          # q positions per attention unit
N_QC = S // QC    # 4 (also the ReduceScatter chunk count)
N_KT = S // 128   # 16 k tiles per attention unit
NO = HID // 512   # 5 o_proj free-dim chunks

_NC_CACHE = None


def _build(reps: int = 1, single: bool = False):
    nc = bacc.Bacc(
        "TRN2", target_bir_lowering=False, debug=False,
        num_devices=(1 if single else NC),
    )

    xt_d = nc.dram_tensor("xt", [N_XL, HC, 128, XL], BF16, kind="ExternalInput").ap()
    wq_d = nc.dram_tensor("wq", [HC, 128, DQ], BF16, kind="ExternalInput").ap()
    wkv_d = nc.dram_tensor("wkv", [HC, 128, 2 * D], BF16, kind="ExternalInput").ap()
    wo_d = nc.dram_tensor("wo", [G, 128, HID], BF16, kind="ExternalInput").ap()
    cwq_d = nc.dram_tensor("cwq", [N_ST, 128, D], F32, kind="ExternalInput").ap()
    swq_d = nc.dram_tensor("swq", [N_ST, 128, D], F32, kind="ExternalInput").ap()
    cwk_d = nc.dram_tensor("cwk", [N_ST, 128, D], F32, kind="ExternalInput").ap()
    swk_d = nc.dram_tensor("swk", [N_ST, 128, D], F32, kind="ExternalInput").ap()
    out_d = nc.dram_tensor("out", [S // NC, HID], BF16, kind="ExternalOutput").ap()

    with tile.TileContext(nc) as tc:
        with (
            tc.tile_pool(name="const", bufs=1) as cpool,
            tc.tile_pool(name="xt", bufs=2) as xt_pool,
            tc.tile_pool(name="cs", bufs=8) as cs_pool,
            tc.tile_pool(name="qw", bufs=5) as qw_pool,
            tc.tile_pool(name="kw", bufs=6) as kw_pool,
            tc.tile_pool(name="ro", bufs=2) as ro_pool,
            tc.tile_pool(name="sm", bufs=4) as sm_pool,
            tc.tile_pool(name="ep", bufs=3) as ep_pool,
            tc.tile_pool(name="ot", bufs=8) as ot_pool,
            tc.tile_pool(name="ob", bufs=2) as ob_pool,
            tc.tile_pool(name="psA", bufs=4, space="PSUM") as psA,
            tc.tile_pool(name="psB", bufs=2, space="PSUM") as psB,
            tc.tile_pool(name="psC", bufs=2, space="PSUM") as psC,
            tc.tile_pool(name="dram", bufs=1, space="DRAM") as dram,
        ):
            for _rep in range(reps):
                # ---- resident constants / weights ----
                ident = cpool.tile([128, 128], BF16, tag="ident")
                masks.make_identity(nc, ident[:])
                # all-ones stationary: the sums matmul then yields the softmax
                # denominator replicated across all 128 partitions (free bcast)
                ones_k = cpool.tile([128, 128], BF16, tag="ones_k")
                nc.vector.memset(ones_k[:], 1.0)

                # first X^T tile before the weight block so the PE can start
                # as soon as xt[0] + wq[0] land
                xt_t = xt_pool.tile([128, HC, XL], BF16, tag="xt")
                nc.sync.dma_start(xt_t[:], xt_d[0].rearrange("c p s -> p c s"))

                # per-chunk weight tiles so the first matmul only waits on
                # chunk 0, not the whole 6.5 MB weight load
                wq_t = []
                wkv_t = []
                xt_next = None
                for ch in range(HC):
                    w1 = cpool.tile([128, DQ], BF16, tag=f"wq{ch}")
                    nc.sync.dma_start(w1[:], wq_d[ch])
                    wq_t.append(w1)
                    w2 = cpool.tile([128, 2 * D], BF16, tag=f"wkv{ch}")
                    nc.sync.dma_start(w2[:], wkv_d[ch])
                    wkv_t.append(w2)
                    if ch == 5:
                        xt_next = xt_pool.tile([128, HC, XL], BF16, tag="xt")
                        nc.sync.dma_start(
                            xt_next[:], xt_d[1].rearrange("c p s -> p c s")
                        )

                qt_sb = cpool.tile([128, G, S], BF16, tag="qt")   # Q^T  [d, h, s]
                kt_sb = cpool.tile([128, S], BF16, tag="kt")      # K^T  [d, s]
                v_sb = cpool.tile([128, N_KT, D], BF16, tag="v")  # V    [s%128, kt, d]

                # ================= phase 1: QKV + norm + rope + transpose ======
                # first attention unit's ep tile; its scores/exp are emitted
                # inside the phase-1 loop as soon as each k-tile's K^T lands
                ep00 = ep_pool.tile([128, N_KT, QC], BF16, tag="ep")
                for st in range(N_ST):
                    if st % (XL // ST) == 0 and st > 0:
                        if st // (XL // ST) == 1:
                            xt_t = xt_next
                        else:
                            xt_t = xt_pool.tile([128, HC, XL], BF16, tag="xt")
                            nc.sync.dma_start(
                                xt_t[:],
                                xt_d[st // (XL // ST)].rearrange("c p s -> p c s"),
                            )
                    soff = (st % (XL // ST)) * ST

                    cwq_t = cs_pool.tile([128, D], F32, tag="cs")
                    nc.sync.dma_start(cwq_t[:], cwq_d[st])
                    swq_t = cs_pool.tile([128, D], F32, tag="cs")
                    nc.sync.dma_start(swq_t[:], swq_d[st])
                    cwk_t = cs_pool.tile([128, D], F32, tag="cs")
                    nc.sync.dma_start(cwk_t[:], cwk_d[st])
                    swk_t = cs_pool.tile([128, D], F32, tag="cs")
                    nc.sync.dma_start(swk_t[:], swk_d[st])

                    q_ps = psA.tile([128, DQ], F32, tag="a")
                    kv_ps = psB.tile([128, 2 * D], F32, tag="b")
                    for ch in range(HC):
                        lhs = xt_t[:, ch, soff : soff + ST]
                        nc.tensor.matmul(
                            q_ps[:], lhs, wq_t[ch][:],
                            start=(ch == 0), stop=(ch == HC - 1),
                        )
                        nc.tensor.matmul(
                            kv_ps[:], lhs, wkv_t[ch][:],
                            start=(ch == 0), stop=(ch == HC - 1),
                        )

                    # evictions (scalar engine)
                    q_sb = qw_pool.tile([128, DQ], F32, tag="qw")
                    nc.scalar.copy(q_sb[:], q_ps[:])
                    k_sb = kw_pool.tile([128, D], F32, tag="kw")
                    nc.scalar.copy(k_sb[:], kv_ps[:, 0:D])
                    nc.scalar.copy(v_sb[:, st, :], kv_ps[:, D : 2 * D])

                    # ---- RMSNorm (per head) ----
                    sq = qw_pool.tile([128, DQ], F32, tag="qw")
                    nc.vector.tensor_mul(sq[:], q_sb[:], q_sb[:])
                    ssq = sm_pool.tile([128, G + 1], F32, tag="sm")
                    nc.vector.tensor_reduce(
                        ssq[:, 0:G], sq[:].rearrange("p (h d) -> p h d", d=D),
                        axis=mybir.AxisListType.X, op=mybir.AluOpType.add,
                    )
                    ksq = kw_pool.tile([128, D], F32, tag="kw")
                    nc.vector.tensor_mul(ksq[:], k_sb[:], k_sb[:])
                    nc.vector.tensor_reduce(
                        ssq[:, G : G + 1], ksq[:].unsqueeze(1),
                        axis=mybir.AxisListType.X, op=mybir.AluOpType.add,
                    )
                    var = sm_pool.tile([128, G + 1], F32, tag="sm")
                    nc.vector.tensor_scalar(
                        var[:], ssq[:], 1.0 / D, EPS,
                        op0=mybir.AluOpType.mult, op1=mybir.AluOpType.add,
                    )
                    rt = sm_pool.tile([128, G + 1], F32, tag="sm")
                    nc.scalar.sqrt(rt[:], var[:])
                    rq = sm_pool.tile([128, G + 1], F32, tag="sm")
                    nc.vector.reciprocal(rq[:], rt[:])
                    rk = rq

                    # ---- normalize + rope (DVE) ----
                    qn = qw_pool.tile([128, DQ], F32, tag="qw")
                    qn3 = qn[:].rearrange("p (h d) -> p h d", d=D)
                    nc.vector.tensor_tensor(
                        qn3, q_sb[:].rearrange("p (h d) -> p h d", d=D),
                        rq[:, 0:G].unsqueeze(2).to_broadcast([128, G, D]),
                        op=mybir.AluOpType.mult,
                    )
                    t1 = qw_pool.tile([128, DQ], F32, tag="qw")
                    t13 = t1[:].rearrange("p (h d) -> p h d", d=D)
                    cwq3 = cwq_t[:].unsqueeze(1).to_broadcast([128, G, D])
                    swq3 = swq_t[:].unsqueeze(1).to_broadcast([128, G, D])
                    nc.vector.tensor_tensor(t13, qn3, cwq3, op=mybir.AluOpType.mult)
                    u = qw_pool.tile([128, DQ], F32, tag="qw")
                    u3 = u[:].rearrange("p (h d) -> p h d", d=D)
                    hd = D // 2
                    nc.vector.tensor_tensor(
                        u3[:, :, 0:hd], qn3[:, :, hd:D], swq3[:, :, 0:hd],
                        op=mybir.AluOpType.mult,
                    )
                    nc.vector.tensor_tensor(
                        u3[:, :, hd:D], qn3[:, :, 0:hd], swq3[:, :, hd:D],
                        op=mybir.AluOpType.mult,
                    )
                    qro = ro_pool.tile([128, DQ], BF16, tag="qro")
                    qro3 = qro[:].rearrange("p (h d) -> p h d", d=D)
                    nc.vector.tensor_sub(qro3[:, :, 0:hd], t13[:, :, 0:hd], u3[:, :, 0:hd])
                    nc.vector.tensor_add(qro3[:, :, hd:D], t13[:, :, hd:D], u3[:, :, hd:D])

                    kn = kw_pool.tile([128, D], F32, tag="kw")
                    nc.vector.tensor_tensor(
                        kn[:], k_sb[:],
                        rk[:, G : G + 1].to_broadcast([128, D]),
                        op=mybir.AluOpType.mult,
                    )
                    kt1 = kw_pool.tile([128, D], F32, tag="kw")
                    nc.vector.tensor_tensor(kt1[:], kn[:], cwk_t[:], op=mybir.AluOpType.mult)
                    ku = kw_pool.tile([128, D], F32, tag="kw")
                    nc.vector.tensor_tensor(
                        ku[:, 0:hd], kn[:, hd:D], swk_t[:, 0:hd], op=mybir.AluOpType.mult
                    )
                    nc.vector.tensor_tensor(
                        ku[:, hd:D], kn[:, 0:hd], swk_t[:, hd:D], op=mybir.AluOpType.mult
                    )
                    kro = ro_pool.tile([128, D], BF16, tag="kro")
                    nc.vector.tensor_sub(kro[:, 0:hd], kt1[:, 0:hd], ku[:, 0:hd])
                    nc.vector.tensor_add(kro[:, hd:D], kt1[:, hd:D], ku[:, hd:D])

                    # ---- transpose Q heads + K into [d, s] ----
                    for h in range(G):
                        tp = psC.tile([128, 128], BF16, tag="c")
                        nc.tensor.transpose(tp[:], qro[:, h * D : (h + 1) * D], ident[:])
                        nc.scalar.copy(qt_sb[:, h, st * ST : (st + 1) * ST], tp[:])
                    tp = psC.tile([128, 128], BF16, tag="c")
                    nc.tensor.transpose(tp[:], kro[:], ident[:])
                    nc.scalar.copy(kt_sb[:, st * ST : (st + 1) * ST], tp[:])

                    if st >= 4:
                        kt = st - 4
                        s_ps = psA.tile([128, QC], F32, tag="a")
                        nc.tensor.matmul(
                            s_ps[:],
                            kt_sb[:, kt * 128 : (kt + 1) * 128],
                            qt_sb[:, 0, 0:QC],
                            start=True, stop=True,
                        )
                        nc.scalar.activation(
                            ep00[:, kt, :], s_ps[:],
                            mybir.ActivationFunctionType.Exp, scale=SCALE,
                        )

                # ================= phase 2: attention + o_proj + RS ============
                # wo is first needed ~10us into phase 2; load it behind the
                # phase-1 traffic instead of ahead of it
                wo_sb = cpool.tile([128, G, HID], BF16, tag="wo")
                nc.sync.dma_start(wo_sb[:], wo_d.rearrange("c p n -> p c n"))
                for qc in range(N_QC):
                    ot_tiles = []
                    for h in range(G):
                        if qc == 0 and h == 0:
                            ep = ep00
                            kt_start = N_ST - 4
                        else:
                            ep = ep_pool.tile([128, N_KT, QC], BF16, tag="ep")
                            kt_start = 0
                        for kt in range(kt_start, N_KT):
                            s_ps = psA.tile([128, QC], F32, tag="a")
                            nc.tensor.matmul(
                                s_ps[:],
                                kt_sb[:, kt * 128 : (kt + 1) * 128],
                                qt_sb[:, h, qc * QC : (qc + 1) * QC],
                                start=True, stop=True,
                            )
                            nc.scalar.activation(
                                ep[:, kt, :], s_ps[:],
                                mybir.ActivationFunctionType.Exp, scale=SCALE,
                            )
                        sums_ps = psC.tile([128, QC], F32, tag="c")
                        pv_ps = psB.tile([128, QC], F32, tag="b")
                        for kt in range(N_KT):
                            nc.tensor.matmul(
                                sums_ps[:], ones_k[:], ep[:, kt, :],
                                start=(kt == 0), stop=(kt == N_KT - 1),
                            )
                            nc.tensor.matmul(
                                pv_ps[:], v_sb[:, kt, :], ep[:, kt, :],
                                start=(kt == 0), stop=(kt == N_KT - 1),
                            )
                        # sums_ps rows are all identical (ones stationary) —
                        # reciprocal gives the denominator broadcast directly
                        rb = sm_pool.tile([128, QC], F32, tag="rb", bufs=2)
                        nc.vector.reciprocal(rb[:], sums_ps[:])
                        ot = ot_pool.tile([128, QC], BF16, tag="ot")
                        nc.vector.tensor_tensor(
                            ot[:], pv_ps[:], rb[:], op=mybir.AluOpType.mult
                        )
                        ot_tiles.append(ot)

                    # o_proj for this 512-row chunk; one full-chunk
                    # ReduceScatter (2.6 MB/rank -> RDH regime, better bus rate)
                    RROWS = QC // NC  # 64 output rows per core per RS
                    if True:
                        rs_in = dram.tile([QC, HID], BF16, tag=f"rsin{qc}")
                        rs_out = dram.tile([RROWS, HID], BF16, tag=f"rsout{qc}")
                        for si in range(QC // ST):
                            sst = si
                            ob = ob_pool.tile([128, HID], BF16, tag="ob")
                            for no in range(NO):
                                y_ps = psB.tile([128, 512], F32, tag="b")
                                for h in range(G):
                                    nc.tensor.matmul(
                                        y_ps[:],
                                        ot_tiles[h][:, sst * ST : (sst + 1) * ST],
                                        wo_sb[:, h, no * 512 : (no + 1) * 512],
                                        start=(h == 0), stop=(h == G - 1),
                                    )
                                # evictions alternate DVE/ACT: the ACT copies
                                # cost ~1.3us table reloads between exp batches,
                                # but measured faster than DVE-only (which
                                # serializes the y_ps rotation)
                                if no % 2 == 0:
                                    nc.vector.tensor_copy(
                                        ob[:, no * 512 : (no + 1) * 512], y_ps[:]
                                    )
                                else:
                                    nc.scalar.copy(
                                        ob[:, no * 512 : (no + 1) * 512], y_ps[:]
                                    )
                                nc.sync.dma_start(
                                    rs_in[si * ST : (si + 1) * ST,
                                          no * 512 : (no + 1) * 512],
                                    ob[:, no * 512 : (no + 1) * 512],
                                )

                        orow = qc * (QC // NC)
                        if single:
                            nc.sync.dma_start(
                                out_d[orow : orow + RROWS, :], rs_in[0:RROWS, :]
                            )
                        else:
                            nc.gpsimd.collective_compute(
                                "ReduceScatter",
                                mybir.AluOpType.add,
                                replica_groups=[list(range(NC))],
                                ins=[rs_in.opt()],
                                outs=[rs_out.opt()],
                            )
                            nc.sync.dma_start(
                                out_d[orow : orow + RROWS, :], rs_out[:]
                            )

    nc.compile()
    return nc


def _get_nc():
    global _NC_CACHE
    if _NC_CACHE is None:
        _NC_CACHE = _build()
    return _NC_CACHE


def make_in_maps(inputs):
    X = np.asarray(inputs["hidden_states"], dtype=np.float32).reshape(S, HID)
    freqs = np.asarray(inputs["freqs_cis"], dtype=np.float32)
    Wq = np.asarray(inputs["Wq"], dtype=np.float32)
    Wk = np.asarray(inputs["Wk"], dtype=np.float32)
    Wv = np.asarray(inputs["Wv"], dtype=np.float32)
    Wo = np.asarray(inputs["Wo"], dtype=np.float32)
    qw = np.asarray(inputs["q_norm_w"], dtype=np.float32)
    kw = np.asarray(inputs["k_norm_w"], dtype=np.float32)

    bf = ml_dtypes.bfloat16
    # X^T load tiles: (L, ch, p, s) = X[L*XL+s, ch*128+p]
    xt = np.ascontiguousarray(
        X.reshape(N_XL, XL, HC, 128).transpose(0, 2, 3, 1).astype(bf)
    )
    cos, sin = freqs[0], freqs[1]  # [S, D]
    cwq = np.ascontiguousarray((cos * qw[None, :]).reshape(N_ST, 128, D))
    swq = np.ascontiguousarray((sin * np.roll(qw, D // 2)[None, :]).reshape(N_ST, 128, D))
    cwk = np.ascontiguousarray((cos * kw[None, :]).reshape(N_ST, 128, D))
    swk = np.ascontiguousarray((sin * np.roll(kw, D // 2)[None, :]).reshape(N_ST, 128, D))

    in_maps = []
    for c in range(NC):
        wq_c = Wq[c * DQ : (c + 1) * DQ, :]  # [DQ, HID]
        wq_t = np.ascontiguousarray(wq_c.T.reshape(HC, 128, DQ).astype(bf))
        wk_c = Wk[c * D : (c + 1) * D, :]
        wv_c = Wv[c * D : (c + 1) * D, :]
        wkv_t = np.ascontiguousarray(
            np.concatenate([wk_c.T, wv_c.T], axis=1).reshape(HC, 128, 2 * D).astype(bf)
        )
        wo_c = Wo[:, c * DQ : (c + 1) * DQ]  # [HID, DQ]
        wo_t = np.ascontiguousarray(wo_c.T.reshape(G, 128, HID).astype(bf))
        in_maps.append(
            {
                "xt": xt,
                "wq": wq_t,
                "wkv": wkv_t,
                "wo": wo_t,
                "cwq": cwq,
                "swq": swq,
                "cwk": cwk,
                "swk": swk,
            }
        )
    return in_maps


def assemble(outs):
    # outs[c] is [S//NC, HID] bf16. RS chunk qc covers global rows
    # [512*qc, +512); core c receives rows [64*c, 64*c+64) of it,
    # stored at core-local rows [64*qc, +64).
    y = np.empty((S, HID), dtype=np.float32)
    rows = QC // NC  # 64
    for qc in range(N_QC):
        for c in range(NC):
            g0 = QC * qc + rows * c
            l0 = rows * qc
            y[g0 : g0 + rows, :] = outs[c][l0 : l0 + rows, :].astype(np.float32)
    return y.reshape(B, S, HID)


def kernel(**inputs) -> np.ndarray:
    nc = _get_nc()
    in_maps = make_in_maps(inputs)
    res = bass_utils.run_bass_kernel_spmd(nc, in_maps, core_ids=list(range(NC)))
    return assemble([r["out"] for r in res.results])



# revision 6
# speedup vs baseline: 1.9598x; 1.9598x over previous
# GQA attention layer (B=1, S=2048, HID=2560, H=32, HKV=8, D=128) on 8 TRN2
# NeuronCores. Tensor-parallel over kv-head groups: core c owns kv head c and
# its 4 query heads (Wq/Wk/Wv row shards, Wo column shard). o_proj partials
# are combined with 4 chunked on-device ReduceScatters over the sequence
# axis, overlapped with compute; the host reassembles the sequence-sharded
# outputs.
#
# Per-core dataflow (all matmuls bf16 -> fp32 PSUM):
#   1. QKV projection from X^T tiles, per-head RMSNorm + RoPE on DVE,
#      PE-transpose of Q/K into [d, s] layout.
#   2. Scores computed transposed (S^T[k, q] = K Q^T), exp on the scalar
#      engine (ACT holds only the Exp table in phase 2; |scores| is bounded
#      so no max subtraction). PV is computed "flipped": lhsT = P^T subtile
#      (stationary), rhs = [V | 1] so the PSUM output is [q, 129] where
#      column 128 is the softmax denominator -- no separate ones-matmul and
#      only a [128,1] reciprocal. The normalized [q, d] tile is PE-transposed
#      to the [d, q] layout o_proj wants.
#   3. o_proj for chunk qc is software-pipelined into chunk qc+1's attention
#      slots; each 512-row chunk ReduceScatters in bf16 while later chunks
#      compute. The rs_out -> out DMAs all sit at the end of the SP queue so
#      no compute DMA ever queues behind a collective.
import sys

if "/opt/trn_rl_repo" not in sys.path:
    sys.path.insert(0, "/opt/trn_rl_repo")

import numpy as np
import ml_dtypes

import concourse.bacc as bacc
import concourse.mybir as mybir
import concourse.tile as tile
from concourse import bass_utils, masks

BF16 = mybir.dt.bfloat16
F32 = mybir.dt.float32

B, S, HID = 1, 2048, 2560
H, HKV, D = 32, 8, 128
G = H // HKV  # q heads per kv head (= per core)
NC = 8  # cores
DQ = G * D  # per-core q width (512)
EPS = 1e-6
SCALE = 1.0 / float(np.sqrt(D))

ST = 128          # s positions per compute tile
N_ST = S // ST    # 16
HC = HID // 128   # 20 contraction chunks
XL = 256          # s positions per X^T DMA load tile
N_XL = S // XL    # 8
QC = 512          # q positions per attention unit
N_QC = S // QC    # 4 (also the ReduceScatter chunk count)
N_KT = S // 128   # 16 k tiles per attention unit
NO = HID // 512   # 5 o_proj free-dim chunks
RROWS = QC // NC  # 64 output rows per core per RS chunk

_NC_CACHE = None


def _build(reps: int = 1, single: bool = False):
    nc = bacc.Bacc(
        "TRN2", target_bir_lowering=False, debug=False,
        num_devices=(1 if single else NC),
    )

    # all host-side layouts are partition-major so every DMA is contiguous
    xt_d = nc.dram_tensor("xt", [N_XL, 128, HC, XL], BF16, kind="ExternalInput").ap()
    wq_d = nc.dram_tensor("wq", [HC, 128, DQ], BF16, kind="ExternalInput").ap()
    wkv_d = nc.dram_tensor("wkv", [HC, 128, 2 * D], BF16, kind="ExternalInput").ap()
    wo_d = nc.dram_tensor("wo", [128, G, HID], BF16, kind="ExternalInput").ap()
    cwq_d = nc.dram_tensor("cwq", [128, N_ST, D], F32, kind="ExternalInput").ap()
    swq_d = nc.dram_tensor("swq", [128, N_ST, D], F32, kind="ExternalInput").ap()
    cwk_d = nc.dram_tensor("cwk", [128, N_ST, D], F32, kind="ExternalInput").ap()
    swk_d = nc.dram_tensor("swk", [128, N_ST, D], F32, kind="ExternalInput").ap()
    out_d = nc.dram_tensor("out", [S // NC, HID], BF16, kind="ExternalOutput").ap()

    with tile.TileContext(nc) as tc:
        with (
            tc.tile_pool(name="const", bufs=1) as cpool,
            tc.tile_pool(name="xt", bufs=2) as xt_pool,
            tc.tile_pool(name="qw", bufs=5) as qw_pool,
            tc.tile_pool(name="kw", bufs=6) as kw_pool,
            tc.tile_pool(name="ro", bufs=2) as ro_pool,
            tc.tile_pool(name="sm", bufs=4) as sm_pool,
            tc.tile_pool(name="ep", bufs=2) as ep_pool,
            tc.tile_pool(name="os", bufs=3) as os_pool,
            tc.tile_pool(name="oT", bufs=2) as oT_pool,
            tc.tile_pool(name="ob", bufs=2) as ob_pool,
            tc.tile_pool(name="psA", bufs=2, space="PSUM") as psA,  # scores
            tc.tile_pool(name="psB", bufs=2, space="PSUM") as psB,  # pv / kv
            tc.tile_pool(name="psC", bufs=2, space="PSUM") as psC,  # transposes
            tc.tile_pool(name="psD", bufs=2, space="PSUM") as psD,  # q / o_proj
            tc.tile_pool(name="dram", bufs=1, space="DRAM") as dram,
        ):
            for _rep in range(reps):
                ident = cpool.tile([128, 128], BF16, tag="ident")
                masks.make_identity(nc, ident[:])

                # first X^T tile ahead of the weight block so the PE can
                # start as soon as xt[0] + wq[0] land
                xt_t = xt_pool.tile([128, HC, XL], BF16, tag="xt")
                nc.sync.dma_start(xt_t[:], xt_d[0])

                wq_t = []
                wkv_t = []
                xt_next = None
                for ch in range(HC):
                    w1 = cpool.tile([128, DQ], BF16, tag=f"wq{ch}")
                    nc.sync.dma_start(w1[:], wq_d[ch])
                    wq_t.append(w1)
                    w2 = cpool.tile([128, 2 * D], BF16, tag=f"wkv{ch}")
                    nc.sync.dma_start(w2[:], wkv_d[ch])
                    wkv_t.append(w2)
                    if ch == 5:
                        xt_next = xt_pool.tile([128, HC, XL], BF16, tag="xt")
                        nc.sync.dma_start(xt_next[:], xt_d[1])

                # rope tables, one contiguous DMA each
                cwq_t = cpool.tile([128, N_ST, D], F32, tag="cwq")
                nc.sync.dma_start(cwq_t[:], cwq_d)
                swq_t = cpool.tile([128, N_ST, D], F32, tag="swq")
                nc.sync.dma_start(swq_t[:], swq_d)
                cwk_t = cpool.tile([128, N_ST, D], F32, tag="cwk")
                nc.sync.dma_start(cwk_t[:], cwk_d)
                swk_t = cpool.tile([128, N_ST, D], F32, tag="swk")
                nc.sync.dma_start(swk_t[:], swk_d)

                qt_sb = cpool.tile([128, G, S], BF16, tag="qt")   # Q^T  [d, h, s]
                kt_sb = cpool.tile([128, S], BF16, tag="kt")      # K^T  [d, s]
                # V with a ones column appended: [s%128, kt, d + 1]
                v_sb = cpool.tile([128, N_KT, D + 1], BF16, tag="v")
                nc.vector.memset(v_sb[:, :, D : D + 1], 1.0)

                # ================= phase 1: QKV + norm + rope + transpose ======
                for st in range(N_ST):
                    if st % (XL // ST) == 0 and st > 0:
                        if st // (XL // ST) == 1:
                            xt_t = xt_next
                        else:
                            xt_t = xt_pool.tile([128, HC, XL], BF16, tag="xt")
                            nc.sync.dma_start(xt_t[:], xt_d[st // (XL // ST)])
                    soff = (st % (XL // ST)) * ST

                    q_ps = psD.tile([128, DQ], F32, tag="d")
                    kv_ps = psB.tile([128, 2 * D], F32, tag="b")
                    for ch in range(HC):
                        lhs = xt_t[:, ch, soff : soff + ST]
                        nc.tensor.matmul(
                            q_ps[:], lhs, wq_t[ch][:],
                            start=(ch == 0), stop=(ch == HC - 1),
                        )
                        nc.tensor.matmul(
                            kv_ps[:], lhs, wkv_t[ch][:],
                            start=(ch == 0), stop=(ch == HC - 1),
                        )

                    # evictions (scalar engine; Copy lives in every ACT table)
                    q_sb = qw_pool.tile([128, DQ], F32, tag="qw")
                    nc.scalar.copy(q_sb[:], q_ps[:])
                    k_sb = kw_pool.tile([128, D], F32, tag="kw")
                    nc.scalar.copy(k_sb[:], kv_ps[:, 0:D])
                    nc.scalar.copy(v_sb[:, st, 0:D], kv_ps[:, D : 2 * D])

                    # ---- RMSNorm (per head) ----
                    sq = qw_pool.tile([128, DQ], F32, tag="qw")
                    nc.vector.tensor_mul(sq[:], q_sb[:], q_sb[:])
                    ssq = sm_pool.tile([128, G + 1], F32, tag="sm")
                    nc.vector.tensor_reduce(
                        ssq[:, 0:G], sq[:].rearrange("p (h d) -> p h d", d=D),
                        axis=mybir.AxisListType.X, op=mybir.AluOpType.add,
                    )
                    ksq = kw_pool.tile([128, D], F32, tag="kw")
                    nc.vector.tensor_mul(ksq[:], k_sb[:], k_sb[:])
                    nc.vector.tensor_reduce(
                        ssq[:, G : G + 1], ksq[:].unsqueeze(1),
                        axis=mybir.AxisListType.X, op=mybir.AluOpType.add,
                    )
                    var = sm_pool.tile([128, G + 1], F32, tag="sm")
                    nc.vector.tensor_scalar(
                        var[:], ssq[:], 1.0 / D, EPS,
                        op0=mybir.AluOpType.mult, op1=mybir.AluOpType.add,
                    )
                    rt = sm_pool.tile([128, G + 1], F32, tag="sm")
                    nc.scalar.sqrt(rt[:], var[:])
                    rq = sm_pool.tile([128, G + 1], F32, tag="sm")
                    nc.vector.reciprocal(rq[:], rt[:])

                    # ---- normalize + rope (DVE) ----
                    cq = cwq_t[:, st, :]
                    sq_ = swq_t[:, st, :]
                    qn = qw_pool.tile([128, DQ], F32, tag="qw")
                    qn3 = qn[:].rearrange("p (h d) -> p h d", d=D)
                    nc.vector.tensor_tensor(
                        qn3, q_sb[:].rearrange("p (h d) -> p h d", d=D),
                        rq[:, 0:G].unsqueeze(2).to_broadcast([128, G, D]),
                        op=mybir.AluOpType.mult,
                    )
                    t1 = qw_pool.tile([128, DQ], F32, tag="qw")
                    t13 = t1[:].rearrange("p (h d) -> p h d", d=D)
                    cwq3 = cq.unsqueeze(1).to_broadcast([128, G, D])
                    swq3 = sq_.unsqueeze(1).to_broadcast([128, G, D])
                    nc.vector.tensor_tensor(t13, qn3, cwq3, op=mybir.AluOpType.mult)
                    u = qw_pool.tile([128, DQ], F32, tag="qw")
                    u3 = u[:].rearrange("p (h d) -> p h d", d=D)
                    hd = D // 2
                    nc.vector.tensor_tensor(
                        u3[:, :, 0:hd], qn3[:, :, hd:D], swq3[:, :, 0:hd],
                        op=mybir.AluOpType.mult,
                    )
                    nc.vector.tensor_tensor(
                        u3[:, :, hd:D], qn3[:, :, 0:hd], swq3[:, :, hd:D],
                        op=mybir.AluOpType.mult,
                    )
                    qro = ro_pool.tile([128, DQ], BF16, tag="qro")
                    qro3 = qro[:].rearrange("p (h d) -> p h d", d=D)
                    nc.vector.tensor_sub(qro3[:, :, 0:hd], t13[:, :, 0:hd], u3[:, :, 0:hd])
                    nc.vector.tensor_add(qro3[:, :, hd:D], t13[:, :, hd:D], u3[:, :, hd:D])

                    kn = kw_pool.tile([128, D], F32, tag="kw")
                    nc.vector.tensor_tensor(
                        kn[:], k_sb[:],
                        rq[:, G : G + 1].to_broadcast([128, D]),
                        op=mybir.AluOpType.mult,
                    )
                    kt1 = kw_pool.tile([128, D], F32, tag="kw")
                    nc.vector.tensor_tensor(
                        kt1[:], kn[:], cwk_t[:, st, :], op=mybir.AluOpType.mult
                    )
                    ku = kw_pool.tile([128, D], F32, tag="kw")
                    nc.vector.tensor_tensor(
                        ku[:, 0:hd], kn[:, hd:D], swk_t[:, st, 0:hd],
                        op=mybir.AluOpType.mult,
                    )
                    nc.vector.tensor_tensor(
                        ku[:, hd:D], kn[:, 0:hd], swk_t[:, st, hd:D],
                        op=mybir.AluOpType.mult,
                    )
                    kro = ro_pool.tile([128, D], BF16, tag="kro")
                    nc.vector.tensor_sub(kro[:, 0:hd], kt1[:, 0:hd], ku[:, 0:hd])
                    nc.vector.tensor_add(kro[:, hd:D], kt1[:, hd:D], ku[:, hd:D])

                    # ---- transpose Q heads + K into [d, s] ----
                    for h in range(G):
                        tp = psC.tile([128, 128], BF16, tag="c")
                        nc.tensor.transpose(tp[:], qro[:, h * D : (h + 1) * D], ident[:])
                        nc.scalar.copy(qt_sb[:, h, st * ST : (st + 1) * ST], tp[:])
                    tp = psC.tile([128, 128], BF16, tag="c")
                    nc.tensor.transpose(tp[:], kro[:], ident[:])
                    nc.scalar.copy(kt_sb[:, st * ST : (st + 1) * ST], tp[:])

                # ================= phase 2: attention + o_proj + RS ============
                # wo is first needed well into phase 2; load it behind the
                # phase-1 traffic instead of ahead of it
                wo_sb = cpool.tile([128, G, HID], BF16, tag="wo")
                nc.sync.dma_start(wo_sb[:], wo_d)

                # software pipeline over the 16 (qc, h) pairs:
                #   slot j of pair i: scores/exp for (i, kt=j), flipped-PV for
                #   pair i-1 (sub j//4, four kt per slot), o_proj units of
                #   chunk qc-1 spread across chunk qc's 64 slots.
                pairs = [(qc, h) for qc in range(N_QC) for h in range(G)]
                state = {}  # pair -> (ep tile, otT tile rotates per qc)
                otT = {}    # qc -> SBUF tile [128, G*4*128] of O^T subtiles
                rs_out_tiles = []

                def emit_scores_slot(pair, ep, j):
                    qc, h = pair
                    s_ps = psA.tile([128, QC], F32, tag="a")
                    nc.tensor.matmul(
                        s_ps[:],
                        kt_sb[:, j * 128 : (j + 1) * 128],
                        qt_sb[:, h, qc * QC : (qc + 1) * QC],
                        start=True, stop=True,
                    )
                    nc.scalar.activation(
                        ep[:, j, :], s_ps[:],
                        mybir.ActivationFunctionType.Exp, scale=SCALE,
                    )

                def emit_pv_slot(pair, ep, j):
                    # four kt of sub-tile j//4; on t==3 the accumulator
                    # finishes: reciprocal + normalize + transpose + evict
                    qc, h = pair
                    sub = j // 4
                    t = j % 4
                    key = ("pv", pair)
                    if t == 0:
                        # same tag/shape as the phase-1 kv tile so the pool
                        # ring stays homogeneous; only [:, 0:D+1] is used
                        state[key] = psB.tile([128, 2 * D], F32, tag="b", name="pv_ps")
                    pv_ps = state[key]
                    for k in range(4):
                        kt = 4 * t + k
                        nc.tensor.matmul(
                            pv_ps[:, 0 : D + 1],
                            ep[:, kt, sub * 128 : (sub + 1) * 128],
                            v_sb[:, kt, :],
                            start=(kt == 0), stop=(kt == N_KT - 1),
                        )
                    if t == 3:
                        rcp = sm_pool.tile([128, 1], F32, tag="rc", bufs=2)
                        nc.vector.reciprocal(rcp[:], pv_ps[:, D : D + 1])
                        o_sb = os_pool.tile([128, 128], BF16, tag="os")
                        nc.vector.tensor_scalar_mul(
                            o_sb[:], pv_ps[:, 0:D], rcp[:, 0:1]
                        )
                        tp = psC.tile([128, 128], BF16, tag="c")
                        nc.tensor.transpose(tp[:], o_sb[:], ident[:])
                        nc.scalar.copy(
                            otT[qc][:, (h * 4 + sub) * 128 : (h * 4 + sub + 1) * 128],
                            tp[:],
                        )

                def emit_oproj_unit(qc, u):
                    # u = si * NO + no
                    si, no = divmod(u, NO)
                    key = ("ob", qc, si)
                    if no == 0:
                        state[key] = ob_pool.tile([128, HID], BF16, tag="ob", name="ob")
                    ob = state[key]
                    y_ps = psD.tile([128, 512], F32, tag="d")
                    for h in range(G):
                        nc.tensor.matmul(
                            y_ps[:],
                            otT[qc][:, (h * 4 + si) * 128 : (h * 4 + si + 1) * 128],
                            wo_sb[:, h, no * 512 : (no + 1) * 512],
                            start=(h == 0), stop=(h == G - 1),
                        )
                    nc.vector.tensor_copy(ob[:, no * 512 : (no + 1) * 512], y_ps[:])
                    if no == NO - 1:
                        rs_in = state[("rsin", qc)]
                        nc.sync.dma_start(
                            rs_in[si * ST : (si + 1) * ST, :], ob[:]
                        )
                        if si == 3:
                            rs_out = dram.tile([RROWS, HID], BF16, tag=f"rsout{qc}")
                            if single:
                                nc.sync.dma_start(rs_out[:], rs_in[0:RROWS, :])
                            else:
                                nc.gpsimd.collective_compute(
                                    "ReduceScatter",
                                    mybir.AluOpType.add,
                                    replica_groups=[list(range(NC))],
                                    ins=[rs_in.opt()],
                                    outs=[rs_out.opt()],
                                )
                            rs_out_tiles.append((qc, rs_out))

                for i in range(len(pairs) + 1):
                    cur = pairs[i] if i < len(pairs) else None
                    prev = pairs[i - 1] if i > 0 else None
                    if cur is not None:
                        qc, h = cur
                        if h == 0:
                            otT[qc] = oT_pool.tile([128, G * 4 * 128], BF16, tag="oT", name="otT")
                            state[("rsin", qc)] = dram.tile(
                                [QC, HID], BF16, tag=f"rsin{qc}", name="rsin"
                            )
                        ep = ep_pool.tile([128, N_KT, QC], BF16, tag="ep")
                        state[("ep", cur)] = ep
                    # o_proj for chunk pqc-1 may only start once otT[pqc-1] is
                    # fully written (end of prev == (pqc, 0)), so stripe its 20
                    # units over the 48 slots of prev == (pqc, 1..3)
                    for j in range(N_KT):
                        if prev is not None:
                            emit_pv_slot(prev, state[("ep", prev)], j)
                            pqc, ph = prev
                            if pqc >= 1 and ph >= 1:
                                sidx = (ph - 1) * N_KT + j
                                lo = sidx * (NO * 4) // 48
                                hi = (sidx + 1) * (NO * 4) // 48
                                for u in range(lo, hi):
                                    emit_oproj_unit(pqc - 1, u)
                        if cur is not None:
                            emit_scores_slot(cur, state[("ep", cur)], j)
                    if prev is not None:
                        del state[("ep", prev)]

                # drain: last chunk's o_proj
                for u in range(4 * NO):
                    emit_oproj_unit(N_QC - 1, u)

                # all output DMAs at the very end of the SP queue
                for qc, rs_out in rs_out_tiles:
                    nc.sync.dma_start(
                        out_d[qc * RROWS : (qc + 1) * RROWS, :], rs_out[:]
                    )

    nc.compile()
    return nc


def _get_nc():
    global _NC_CACHE
    if _NC_CACHE is None:
        _NC_CACHE = _build()
    return _NC_CACHE


def make_in_maps(inputs):
    X = np.asarray(inputs["hidden_states"], dtype=np.float32).reshape(S, HID)
    freqs = np.asarray(inputs["freqs_cis"], dtype=np.float32)
    Wq = np.asarray(inputs["Wq"], dtype=np.float32)
    Wk = np.asarray(inputs["Wk"], dtype=np.float32)
    Wv = np.asarray(inputs["Wv"], dtype=np.float32)
    Wo = np.asarray(inputs["Wo"], dtype=np.float32)
    qw = np.asarray(inputs["q_norm_w"], dtype=np.float32)
    kw = np.asarray(inputs["k_norm_w"], dtype=np.float32)

    bf = ml_dtypes.bfloat16
    # X^T load tiles, partition-major: xt[L, p, ch, s] = X[L*XL+s, ch*128+p]
    xt = np.ascontiguousarray(
        X.reshape(N_XL, XL, HC, 128).transpose(0, 3, 2, 1).astype(bf)
    )
    cos, sin = freqs[0], freqs[1]  # [S, D]
    # rope tables, partition-major: t[p, st, d] = table[st*128 + p, d]
    cwq = np.ascontiguousarray(
        (cos * qw[None, :]).reshape(N_ST, 128, D).transpose(1, 0, 2)
    )
    swq = np.ascontiguousarray(
        (sin * np.roll(qw, D // 2)[None, :]).reshape(N_ST, 128, D).transpose(1, 0, 2)
    )
    cwk = np.ascontiguousarray(
        (cos * kw[None, :]).reshape(N_ST, 128, D).transpose(1, 0, 2)
    )
    swk = np.ascontiguousarray(
        (sin * np.roll(kw, D // 2)[None, :]).reshape(N_ST, 128, D).transpose(1, 0, 2)
    )

    in_maps = []
    for c in range(NC):
        wq_c = Wq[c * DQ : (c + 1) * DQ, :]  # [DQ, HID]
        wq_t = np.ascontiguousarray(wq_c.T.reshape(HC, 128, DQ).astype(bf))
        wk_c = Wk[c * D : (c + 1) * D, :]
        wv_c = Wv[c * D : (c + 1) * D, :]
        wkv_t = np.ascontiguousarray(
            np.concatenate([wk_c.T, wv_c.T], axis=1).reshape(HC, 128, 2 * D).astype(bf)
        )
        wo_c = Wo[:, c * DQ : (c + 1) * DQ]  # [HID, DQ]
        # partition-major: wo[p, g, n] = Wo[n, c*DQ + g*128 + p]
        wo_t = np.ascontiguousarray(
            wo_c.T.reshape(G, 128, HID).transpose(1, 0, 2).astype(bf)
        )
        in_maps.append(
            {
                "xt": xt,
                "wq": wq_t,
                "wkv": wkv_t,
                "wo": wo_t,
                "cwq": cwq,
                "swq": swq,
                "cwk": cwk,
                "swk": swk,
            }
        )
    return in_maps


def assemble(outs):
    # outs[c] is [S//NC, HID] bf16. RS chunk qc covers global rows
    # [512*qc, +512); core c receives rows [64*c, 64*c+64) of it,
    # stored at core-local rows [64*qc, +64).
    y = np.empty((S, HID), dtype=np.float32)
    for qc in range(N_QC):
        for c in range(NC):
            g0 = QC * qc + RROWS * c
            l0 = RROWS * qc
            y[g0 : g0 + RROWS, :] = outs[c][l0 : l0 + RROWS, :].astype(np.float32)
    return y.reshape(B, S, HID)


def kernel(**inputs) -> np.ndarray:
    nc = _get_nc()
    in_maps = make_in_maps(inputs)
    res = bass_utils.run_bass_kernel_spmd(nc, in_maps, core_ids=list(range(NC)))
    return assemble([r["out"] for r in res.results])


# revision 9
# speedup vs baseline: 2.1505x; 1.0973x over previous
# GQA attention layer (B=1, S=2048, HID=2560, H=32, HKV=8, D=128) on 8 TRN2
# NeuronCores. Tensor-parallel over kv-head groups: core c owns kv head c and
# its 4 query heads (Wq/Wk/Wv row shards, Wo column shard). o_proj partials
# are combined with 4 chunked on-device ReduceScatters over the sequence
# axis, overlapped with compute; the host reassembles the sequence-sharded
# outputs.
#
# Per-core dataflow (all matmuls bf16 -> fp32 PSUM):
#   1. QKV projection from X^T tiles, per-head RMSNorm + RoPE on DVE,
#      PE-transpose of Q/K into [d, s] layout.
#   2. Scores computed transposed (S^T[k, q] = K Q^T), exp on the scalar
#      engine (ACT holds only the Exp table in phase 2; |scores| is bounded
#      so no max subtraction). PV is computed "flipped": lhsT = P^T subtile
#      (stationary), rhs = [V | 1] so the PSUM output is [q, 129] where
#      column 128 is the softmax denominator -- no separate ones-matmul and
#      only a [128,1] reciprocal. The normalized [q, d] tile is PE-transposed
#      to the [d, q] layout o_proj wants.
#   3. o_proj for chunk qc is software-pipelined into chunk qc+1's attention
#      slots; each 512-row chunk ReduceScatters in bf16 while later chunks
#      compute. The rs_out -> out DMAs all sit at the end of the SP queue so
#      no compute DMA ever queues behind a collective.
import sys

if "/opt/trn_rl_repo" not in sys.path:
    sys.path.insert(0, "/opt/trn_rl_repo")

import numpy as np
import ml_dtypes

import concourse.bacc as bacc
import concourse.mybir as mybir
import concourse.tile as tile
from concourse import bass_utils, masks

BF16 = mybir.dt.bfloat16
F32 = mybir.dt.float32

B, S, HID = 1, 2048, 2560
H, HKV, D = 32, 8, 128
G = H // HKV  # q heads per kv head (= per core)
NC = 8  # cores
DQ = G * D  # per-core q width (512)
EPS = 1e-6
SCALE = 1.0 / float(np.sqrt(D))

ST = 128          # s positions per compute tile
N_ST = S // ST    # 16
HC = HID // 128   # 20 contraction chunks
XL = 256          # s positions per X^T DMA load tile
N_XL = S // XL    # 8
QC = 512          # q positions per attention unit
N_QC = S // QC    # 4 (also the ReduceScatter chunk count)
N_KT = S // 128   # 16 k tiles per attention unit
NO = HID // 512   # 5 o_proj free-dim chunks
RROWS = QC // NC  # 64 output rows per core per RS chunk

_NC_CACHE = None


def _build(reps: int = 1, single: bool = False):
    nc = bacc.Bacc(
        "TRN2", target_bir_lowering=False, debug=False,
        num_devices=(1 if single else NC),
    )

    # all host-side layouts are partition-major so every DMA is contiguous
    xt_d = nc.dram_tensor("xt", [N_XL, 128, HC, XL], BF16, kind="ExternalInput").ap()
    wq_d = nc.dram_tensor("wq", [HC, 128, DQ], BF16, kind="ExternalInput").ap()
    wkv_d = nc.dram_tensor("wkv", [HC, 128, 2 * D], BF16, kind="ExternalInput").ap()
    wo_d = nc.dram_tensor("wo", [128, G, HID], BF16, kind="ExternalInput").ap()
    cwq_d = nc.dram_tensor("cwq", [128, N_ST, D], F32, kind="ExternalInput").ap()
    swq_d = nc.dram_tensor("swq", [128, N_ST, D], F32, kind="ExternalInput").ap()
    cwk_d = nc.dram_tensor("cwk", [128, N_ST, D], F32, kind="ExternalInput").ap()
    swk_d = nc.dram_tensor("swk", [128, N_ST, D], F32, kind="ExternalInput").ap()
    out_d = nc.dram_tensor("out", [S // NC, HID], BF16, kind="ExternalOutput").ap()

    with tile.TileContext(nc) as tc:
        with (
            tc.tile_pool(name="const", bufs=1) as cpool,
            tc.tile_pool(name="xt", bufs=2) as xt_pool,
            tc.tile_pool(name="qw", bufs=5) as qw_pool,
            tc.tile_pool(name="kw", bufs=6) as kw_pool,
            tc.tile_pool(name="ro", bufs=2) as ro_pool,
            tc.tile_pool(name="sm", bufs=4) as sm_pool,
            tc.tile_pool(name="ep", bufs=2) as ep_pool,
            tc.tile_pool(name="os", bufs=3) as os_pool,
            tc.tile_pool(name="oT", bufs=2) as oT_pool,
            tc.tile_pool(name="ob", bufs=2) as ob_pool,
            tc.tile_pool(name="psA", bufs=2, space="PSUM") as psA,  # scores
            tc.tile_pool(name="psB", bufs=2, space="PSUM") as psB,  # pv / kv
            tc.tile_pool(name="psC", bufs=2, space="PSUM") as psC,  # transposes
            tc.tile_pool(name="psD", bufs=2, space="PSUM") as psD,  # q / o_proj
            tc.tile_pool(name="dram", bufs=1, space="DRAM") as dram,
        ):
            for _rep in range(reps):
                ident = cpool.tile([128, 128], BF16, tag="ident")
                masks.make_identity(nc, ident[:])

                # first X^T tile ahead of the weight block so the PE can
                # start as soon as xt[0] + wq[0] land
                xt_t = xt_pool.tile([128, HC, XL], BF16, tag="xt")
                nc.sync.dma_start(xt_t[:], xt_d[0])

                wq_t = []
                wkv_t = []
                xt_next = None
                for ch in range(HC):
                    w1 = cpool.tile([128, DQ], BF16, tag=f"wq{ch}")
                    nc.sync.dma_start(w1[:], wq_d[ch])
                    wq_t.append(w1)
                    w2 = cpool.tile([128, 2 * D], BF16, tag=f"wkv{ch}")
                    nc.sync.dma_start(w2[:], wkv_d[ch])
                    wkv_t.append(w2)
                    if ch == 5:
                        xt_next = xt_pool.tile([128, HC, XL], BF16, tag="xt")
                        nc.sync.dma_start(xt_next[:], xt_d[1])

                # rope tables, one contiguous DMA each
                cwq_t = cpool.tile([128, N_ST, D], F32, tag="cwq")
                nc.sync.dma_start(cwq_t[:], cwq_d)
                swq_t = cpool.tile([128, N_ST, D], F32, tag="swq")
                nc.sync.dma_start(swq_t[:], swq_d)
                cwk_t = cpool.tile([128, N_ST, D], F32, tag="cwk")
                nc.sync.dma_start(cwk_t[:], cwk_d)
                swk_t = cpool.tile([128, N_ST, D], F32, tag="swk")
                nc.sync.dma_start(swk_t[:], swk_d)

                qt_sb = cpool.tile([128, G, S], BF16, tag="qt")   # Q^T  [d, h, s]
                kt_sb = cpool.tile([128, S], BF16, tag="kt")      # K^T  [d, s]
                # V with a ones column appended: [s%128, kt, d + 1]
                v_sb = cpool.tile([128, N_KT, D + 1], BF16, tag="v")
                nc.vector.memset(v_sb[:, :, D : D + 1], 1.0)

                # ================= phase 1: QKV + norm + rope + transpose ======
                for st in range(N_ST):
                    if st % (XL // ST) == 0 and st > 0:
                        if st // (XL // ST) == 1:
                            xt_t = xt_next
                        else:
                            xt_t = xt_pool.tile([128, HC, XL], BF16, tag="xt")
                            nc.sync.dma_start(xt_t[:], xt_d[st // (XL // ST)])
                    soff = (st % (XL // ST)) * ST

                    q_ps = psD.tile([128, DQ], F32, tag="d")
                    kv_ps = psB.tile([128, 2 * D], F32, tag="b")
                    for ch in range(HC):
                        lhs = xt_t[:, ch, soff : soff + ST]
                        nc.tensor.matmul(
                            q_ps[:], lhs, wq_t[ch][:],
                            start=(ch == 0), stop=(ch == HC - 1),
                        )
                        nc.tensor.matmul(
                            kv_ps[:], lhs, wkv_t[ch][:],
                            start=(ch == 0), stop=(ch == HC - 1),
                        )

                    # evictions (scalar engine; Copy lives in every ACT table)
                    q_sb = qw_pool.tile([128, DQ], F32, tag="qw")
                    nc.scalar.copy(q_sb[:], q_ps[:])
                    k_sb = kw_pool.tile([128, D], F32, tag="kw")
                    nc.scalar.copy(k_sb[:], kv_ps[:, 0:D])
                    nc.scalar.copy(v_sb[:, st, 0:D], kv_ps[:, D : 2 * D])

                    # ---- RMSNorm (per head) ----
                    sq = qw_pool.tile([128, DQ], F32, tag="qw")
                    nc.vector.tensor_mul(sq[:], q_sb[:], q_sb[:])
                    ssq = sm_pool.tile([128, G + 1], F32, tag="sm")
                    nc.vector.tensor_reduce(
                        ssq[:, 0:G], sq[:].rearrange("p (h d) -> p h d", d=D),
                        axis=mybir.AxisListType.X, op=mybir.AluOpType.add,
                    )
                    ksq = kw_pool.tile([128, D], F32, tag="kw")
                    nc.vector.tensor_mul(ksq[:], k_sb[:], k_sb[:])
                    nc.vector.tensor_reduce(
                        ssq[:, G : G + 1], ksq[:].unsqueeze(1),
                        axis=mybir.AxisListType.X, op=mybir.AluOpType.add,
                    )
                    var = sm_pool.tile([128, G + 1], F32, tag="sm")
                    nc.vector.tensor_scalar(
                        var[:], ssq[:], 1.0 / D, EPS,
                        op0=mybir.AluOpType.mult, op1=mybir.AluOpType.add,
                    )
                    rt = sm_pool.tile([128, G + 1], F32, tag="sm")
                    nc.scalar.sqrt(rt[:], var[:])
                    rq = sm_pool.tile([128, G + 1], F32, tag="sm")
                    nc.vector.reciprocal(rq[:], rt[:])

                    # ---- normalize + rope (DVE) ----
                    cq = cwq_t[:, st, :]
                    sq_ = swq_t[:, st, :]
                    qn = qw_pool.tile([128, DQ], F32, tag="qw")
                    qn3 = qn[:].rearrange("p (h d) -> p h d", d=D)
                    nc.vector.tensor_tensor(
                        qn3, q_sb[:].rearrange("p (h d) -> p h d", d=D),
                        rq[:, 0:G].unsqueeze(2).to_broadcast([128, G, D]),
                        op=mybir.AluOpType.mult,
                    )
                    t1 = qw_pool.tile([128, DQ], F32, tag="qw")
                    t13 = t1[:].rearrange("p (h d) -> p h d", d=D)
                    cwq3 = cq.unsqueeze(1).to_broadcast([128, G, D])
                    swq3 = sq_.unsqueeze(1).to_broadcast([128, G, D])
                    nc.vector.tensor_tensor(t13, qn3, cwq3, op=mybir.AluOpType.mult)
                    u = qw_pool.tile([128, DQ], F32, tag="qw")
                    u3 = u[:].rearrange("p (h d) -> p h d", d=D)
                    hd = D // 2
                    nc.vector.tensor_tensor(
                        u3[:, :, 0:hd], qn3[:, :, hd:D], swq3[:, :, 0:hd],
                        op=mybir.AluOpType.mult,
                    )
                    nc.vector.tensor_tensor(
                        u3[:, :, hd:D], qn3[:, :, 0:hd], swq3[:, :, hd:D],
                        op=mybir.AluOpType.mult,
                    )
                    qro = ro_pool.tile([128, DQ], BF16, tag="qro")
                    qro3 = qro[:].rearrange("p (h d) -> p h d", d=D)
                    nc.vector.tensor_sub(qro3[:, :, 0:hd], t13[:, :, 0:hd], u3[:, :, 0:hd])
                    nc.vector.tensor_add(qro3[:, :, hd:D], t13[:, :, hd:D], u3[:, :, hd:D])

                    kn = kw_pool.tile([128, D], F32, tag="kw")
                    nc.vector.tensor_tensor(
                        kn[:], k_sb[:],
                        rq[:, G : G + 1].to_broadcast([128, D]),
                        op=mybir.AluOpType.mult,
                    )
                    kt1 = kw_pool.tile([128, D], F32, tag="kw")
                    nc.vector.tensor_tensor(
                        kt1[:], kn[:], cwk_t[:, st, :], op=mybir.AluOpType.mult
                    )
                    ku = kw_pool.tile([128, D], F32, tag="kw")
                    nc.vector.tensor_tensor(
                        ku[:, 0:hd], kn[:, hd:D], swk_t[:, st, 0:hd],
                        op=mybir.AluOpType.mult,
                    )
                    nc.vector.tensor_tensor(
                        ku[:, hd:D], kn[:, 0:hd], swk_t[:, st, hd:D],
                        op=mybir.AluOpType.mult,
                    )
                    kro = ro_pool.tile([128, D], BF16, tag="kro")
                    nc.vector.tensor_sub(kro[:, 0:hd], kt1[:, 0:hd], ku[:, 0:hd])
                    nc.vector.tensor_add(kro[:, hd:D], kt1[:, hd:D], ku[:, hd:D])

                    # ---- transpose Q heads + K into [d, s] ----
                    for h in range(G):
                        tp = psC.tile([128, 128], BF16, tag="c")
                        nc.tensor.transpose(tp[:], qro[:, h * D : (h + 1) * D], ident[:])
                        nc.scalar.copy(qt_sb[:, h, st * ST : (st + 1) * ST], tp[:])
                    tp = psC.tile([128, 128], BF16, tag="c")
                    nc.tensor.transpose(tp[:], kro[:], ident[:])
                    nc.scalar.copy(kt_sb[:, st * ST : (st + 1) * ST], tp[:])

                # ================= phase 2: attention + o_proj + RS ============
                # wo is first needed well into phase 2; load it behind the
                # phase-1 traffic instead of ahead of it
                wo_sb = cpool.tile([128, G, HID], BF16, tag="wo")
                nc.sync.dma_start(wo_sb[:], wo_d)

                # software pipeline over the 16 (qc, h) pairs:
                #   slot j of pair i: scores/exp for (i, kt=j), flipped-PV for
                #   pair i-1 (sub j//4, four kt per slot), o_proj units of
                #   chunk qc-1 spread across chunk qc's 64 slots.
                pairs = [(qc, h) for qc in range(N_QC) for h in range(G)]
                state = {}  # pair -> (ep tile, otT tile rotates per qc)
                otT = {}    # qc -> SBUF tile [128, G*4*128] of O^T subtiles
                rs_out_tiles = []

                def emit_scores_slot(pair, ep, j):
                    qc, h = pair
                    s_ps = psA.tile([128, QC], F32, tag="a")
                    nc.tensor.matmul(
                        s_ps[:],
                        kt_sb[:, j * 128 : (j + 1) * 128],
                        qt_sb[:, h, qc * QC : (qc + 1) * QC],
                        start=True, stop=True,
                    )
                    nc.scalar.activation(
                        ep[:, j, :], s_ps[:],
                        mybir.ActivationFunctionType.Exp, scale=SCALE,
                    )

                def emit_pv_slot(pair, ep, j):
                    # four kt of sub-tile j//4; on t==3 the accumulator
                    # finishes: reciprocal + normalize + transpose + evict
                    qc, h = pair
                    sub = j // 4
                    t = j % 4
                    key = ("pv", pair)
                    if t == 0:
                        # same tag/shape as the phase-1 kv tile so the pool
                        # ring stays homogeneous; only [:, 0:D+1] is used
                        state[key] = psB.tile([128, 2 * D], F32, tag="b", name="pv_ps")
                    pv_ps = state[key]
                    for k in range(4):
                        kt = 4 * t + k
                        nc.tensor.matmul(
                            pv_ps[:, 0 : D + 1],
                            ep[:, kt, sub * 128 : (sub + 1) * 128],
                            v_sb[:, kt, :],
                            start=(kt == 0), stop=(kt == N_KT - 1),
                        )
                    if t == 3:
                        rcp = sm_pool.tile([128, 1], F32, tag="rc", bufs=2)
                        nc.vector.reciprocal(rcp[:], pv_ps[:, D : D + 1])
                        o_sb = os_pool.tile([128, 128], BF16, tag="os")
                        nc.vector.tensor_scalar_mul(
                            o_sb[:], pv_ps[:, 0:D], rcp[:, 0:1]
                        )
                        tp = psC.tile([128, 128], BF16, tag="c")
                        nc.tensor.transpose(tp[:], o_sb[:], ident[:])
                        nc.scalar.copy(
                            otT[qc][:, (h * 4 + sub) * 128 : (h * 4 + sub + 1) * 128],
                            tp[:],
                        )

                def emit_oproj_unit(qc, u):
                    # u = si * NO + no
                    si, no = divmod(u, NO)
                    key = ("ob", qc, si)
                    if no == 0:
                        state[key] = ob_pool.tile([128, HID], BF16, tag="ob", name="ob")
                    ob = state[key]
                    y_ps = psD.tile([128, 512], F32, tag="d")
                    for h in range(G):
                        nc.tensor.matmul(
                            y_ps[:],
                            otT[qc][:, (h * 4 + si) * 128 : (h * 4 + si + 1) * 128],
                            wo_sb[:, h, no * 512 : (no + 1) * 512],
                            start=(h == 0), stop=(h == G - 1),
                        )
                    nc.vector.tensor_copy(ob[:, no * 512 : (no + 1) * 512], y_ps[:])
                    if no == NO - 1:
                        rs_in = state[("rsin", qc)]
                        nc.sync.dma_start(
                            rs_in[si * ST : (si + 1) * ST, :], ob[:]
                        )
                        if si == 3:
                            rs_out = dram.tile([RROWS, HID], BF16, tag=f"rsout{qc}")
                            if single:
                                nc.sync.dma_start(rs_out[:], rs_in[0:RROWS, :])
                            else:
                                nc.gpsimd.collective_compute(
                                    "ReduceScatter",
                                    mybir.AluOpType.add,
                                    replica_groups=[list(range(NC))],
                                    ins=[rs_in.opt()],
                                    outs=[rs_out.opt()],
                                )
                            rs_out_tiles.append((qc, rs_out))

                for i in range(len(pairs) + 1):
                    cur = pairs[i] if i < len(pairs) else None
                    prev = pairs[i - 1] if i > 0 else None
                    if cur is not None:
                        qc, h = cur
                        if h == 0:
                            otT[qc] = oT_pool.tile([128, G * 4 * 128], BF16, tag="oT", name="otT")
                            state[("rsin", qc)] = dram.tile(
                                [QC, HID], BF16, tag=f"rsin{qc}", name="rsin"
                            )
                        ep = ep_pool.tile([128, N_KT, QC], BF16, tag="ep")
                        state[("ep", cur)] = ep
                    # o_proj for chunk pqc-1 may only start once otT[pqc-1] is
                    # fully written (end of prev == (pqc, 0)), so stripe its 20
                    # units over the 48 slots of prev == (pqc, 1..3)
                    for j in range(N_KT):
                        if prev is not None:
                            emit_pv_slot(prev, state[("ep", prev)], j)
                            pqc, ph = prev
                            if pqc >= 1 and ph >= 1:
                                sidx = (ph - 1) * N_KT + j
                                lo = sidx * (NO * 4) // 48
                                hi = (sidx + 1) * (NO * 4) // 48
                                for u in range(lo, hi):
                                    emit_oproj_unit(pqc - 1, u)
                        if cur is not None:
                            emit_scores_slot(cur, state[("ep", cur)], j)
                    if prev is not None:
                        del state[("ep", prev)]

                # drain: last chunk's o_proj
                for u in range(4 * NO):
                    emit_oproj_unit(N_QC - 1, u)

                # output DMAs on the gpsimd queue: they wait on collective
                # completion, and the scheduler may hoist them -- on the SP
                # queue that head-of-line blocks compute DMAs behind the RS,
                # while on the gpsimd queue it only delays cc triggers that
                # are serialized behind the same RS anyway
                for qc, rs_out in rs_out_tiles:
                    nc.gpsimd.dma_start(
                        out_d[qc * RROWS : (qc + 1) * RROWS, :], rs_out[:]
                    )

    nc.compile()
    return nc


def _get_nc():
    global _NC_CACHE
    if _NC_CACHE is None:
        _NC_CACHE = _build()
    return _NC_CACHE


def make_in_maps(inputs):
    X = np.asarray(inputs["hidden_states"], dtype=np.float32).reshape(S, HID)
    freqs = np.asarray(inputs["freqs_cis"], dtype=np.float32)
    Wq = np.asarray(inputs["Wq"], dtype=np.float32)
    Wk = np.asarray(inputs["Wk"], dtype=np.float32)
    Wv = np.asarray(inputs["Wv"], dtype=np.float32)
    Wo = np.asarray(inputs["Wo"], dtype=np.float32)
    qw = np.asarray(inputs["q_norm_w"], dtype=np.float32)
    kw = np.asarray(inputs["k_norm_w"], dtype=np.float32)

    bf = ml_dtypes.bfloat16
    # X^T load tiles, partition-major: xt[L, p, ch, s] = X[L*XL+s, ch*128+p]
    xt = np.ascontiguousarray(
        X.reshape(N_XL, XL, HC, 128).transpose(0, 3, 2, 1).astype(bf)
    )
    cos, sin = freqs[0], freqs[1]  # [S, D]
    # rope tables, partition-major: t[p, st, d] = table[st*128 + p, d]
    cwq = np.ascontiguousarray(
        (cos * qw[None, :]).reshape(N_ST, 128, D).transpose(1, 0, 2)
    )
    swq = np.ascontiguousarray(
        (sin * np.roll(qw, D // 2)[None, :]).reshape(N_ST, 128, D).transpose(1, 0, 2)
    )
    cwk = np.ascontiguousarray(
        (cos * kw[None, :]).reshape(N_ST, 128, D).transpose(1, 0, 2)
    )
    swk = np.ascontiguousarray(
        (sin * np.roll(kw, D // 2)[None, :]).reshape(N_ST, 128, D).transpose(1, 0, 2)
    )

    in_maps = []
    for c in range(NC):
        wq_c = Wq[c * DQ : (c + 1) * DQ, :]  # [DQ, HID]
        wq_t = np.ascontiguousarray(wq_c.T.reshape(HC, 128, DQ).astype(bf))
        wk_c = Wk[c * D : (c + 1) * D, :]
        wv_c = Wv[c * D : (c + 1) * D, :]
        wkv_t = np.ascontiguousarray(
            np.concatenate([wk_c.T, wv_c.T], axis=1).reshape(HC, 128, 2 * D).astype(bf)
        )
        wo_c = Wo[:, c * DQ : (c + 1) * DQ]  # [HID, DQ]
        # partition-major: wo[p, g, n] = Wo[n, c*DQ + g*128 + p]
        wo_t = np.ascontiguousarray(
            wo_c.T.reshape(G, 128, HID).transpose(1, 0, 2).astype(bf)
        )
        in_maps.append(
            {
                "xt": xt,
                "wq": wq_t,
                "wkv": wkv_t,
                "wo": wo_t,
                "cwq": cwq,
                "swq": swq,
                "cwk": cwk,
                "swk": swk,
            }
        )
    return in_maps


def assemble(outs):
    # outs[c] is [S//NC, HID] bf16. RS chunk qc covers global rows
    # [512*qc, +512); core c receives rows [64*c, 64*c+64) of it,
    # stored at core-local rows [64*qc, +64).
    y = np.empty((S, HID), dtype=np.float32)
    for qc in range(N_QC):
        for c in range(NC):
            g0 = QC * qc + RROWS * c
            l0 = RROWS * qc
            y[g0 : g0 + RROWS, :] = outs[c][l0 : l0 + RROWS, :].astype(np.float32)
    return y.reshape(B, S, HID)


def kernel(**inputs) -> np.ndarray:
    nc = _get_nc()
    in_maps = make_in_maps(inputs)
    res = bass_utils.run_bass_kernel_spmd(nc, in_maps, core_ids=list(range(NC)))
    return assemble([r["out"] for r in res.results])
